# revision 19
# baseline (speedup 1.0000x reference)
"""Dot-product attention with per-batch key masking on 8 TRN2 NeuronCores.

Problem: B=8, S=4096, D=64 fp32.
  scores = (Q @ K^T) / sqrt(D); scores[:, :, k >= valid_len[b]] = -1e9
  attn = softmax(scores, axis=-1);  out = attn @ V
Returns (out [B,S,D], attn [B,S,S]) like the reference.

Sharding: batch across the 8 cores (data parallel), one batch element per core.

Per-core device algorithm (everything in TRANSPOSED [k, q] layout):
  - Host supplies Q^T/K^T split into fp32r hi/lo parts. Scores are computed
    in TWO fp32r matmul passes accumulating in PSUM:
      pass1: [Khi; maskrow] . [Qhi; ones]   (the mask-bias row adds -1e12
             to masked k columns inside the matmul)
      pass2: [Khi; Klo] . [Qlo; Qhi]        (cross terms)
    leaving only ~2^-26-relative error (fp32r rounds mantissas to 12 bits;
    single-pass fp32r would give ~1e-4).
  - exp on ScalarE with scale=1/sqrt(D) folded in: pT = exp(sT/8).
    Masked rows underflow to exactly 0.0, matching the reference's
    masked_fill(-1e9) -> softmax behavior exactly.
  - AV matmul with V augmented by a ones column: out_aug[0:64, q] =
    sum_k pT[k,q] V[k,d]; out_aug[64, q] = sum_k pT[k,q] = softmax
    denominator. Accumulates in PSUM over the 32 k-tiles.
  - attn normalization: recip = 1/denom (VectorE, exact fp32), replicated
    across partitions via a DRAM-bounce broadcast DMA, then pT *= recip
    on VectorE; stores to HBM as contiguous [kt, qb, 128, 512] tiles.
  - Host reassembles attn (pure numpy transpose) and computes
    out = (out_aug[:64] / denom).T.
"""

import numpy as np

B, S, D = 8, 4096, 64
DA = D + 1          # augmented contraction: 64 data rows + 1 mask/ones row
KT = 128            # k-tile (partition dim of sT / contraction tile for AV)
NKT = S // KT       # 32
QB = 512            # q-block (moving free dim, = one PSUM bank of fp32)
NQB = S // QB       # 8
PAIR = 2            # k-tiles per PSUM superblock -> exp granularity [128, 1024]
NPAIR = NKT // PAIR # 16
NEG_BIAS = -1e12    # added to masked raw scores; exp((s-1e12)/8) == 0.0
SCALE = 1.0 / 8.0   # 1/sqrt(64)

_cache = {}


def _round_fp32r(x):
    """fp32 -> fp32r: round-to-nearest-even to 12 mantissa bits (matches
    libwalrus fp32_to_fp32r and TRN2 PE behavior, verified on HW)."""
    b = np.ascontiguousarray(x, np.float32).view(np.uint32)
    r = (b + 0x7FF + ((b >> 12) & 1)) & 0xFFFFF000
    return r.view(np.float32)


def _build_bass():
    import concourse.bass as bass
    import concourse.mybir as mybir
    import concourse.tile as tile
    from concourse import bacc

    F32 = mybir.dt.float32
    F32R = mybir.dt.float32r
    EXP = mybir.ActivationFunctionType.Exp

    nc = bacc.Bacc("TRN2", target_bir_lowering=False)

    qt_d = nc.dram_tensor("qt_aug", [DA, S], F32R, kind="ExternalInput").ap()
    kt_d = nc.dram_tensor("kt_aug", [DA, S], F32R, kind="ExternalInput").ap()
    qt2_d = nc.dram_tensor("qt2", [2 * D, S], F32R, kind="ExternalInput").ap()
    kt2_d = nc.dram_tensor("kt2", [2 * D, S], F32R, kind="ExternalInput").ap()
    v_d = nc.dram_tensor("v_aug", [S, DA], F32R, kind="ExternalInput").ap()
    # attn^T stored as contiguous [kt, qb, 128, 512] tiles for DMA efficiency
    attn_d = nc.dram_tensor(
        "attn_t", [NKT, NQB, KT, QB], F32R, kind="ExternalOutput"
    ).ap()
    outd_d = nc.dram_tensor("outd", [DA, S], F32, kind="ExternalOutput").ap()

    with tile.TileContext(nc) as tc:
        with (
            tc.tile_pool(name="const", bufs=1) as const,
            tc.tile_pool(name="ptp", bufs=24) as ptp,
            tc.tile_pool(name="small", bufs=2) as small,
            tc.tile_pool(name="sps", bufs=3, space="PSUM") as sps,
            tc.tile_pool(name="ops", bufs=2, space="PSUM") as ops,
            tc.tile_pool(name="drp", bufs=2, space="DRAM") as drp,
        ):
            qt_sb = const.tile([DA, S], F32R)
            kt_sb = const.tile([DA, S], F32R)
            qt2_sb = const.tile([2 * D, S], F32R)
            kt2_sb = const.tile([2 * D, S], F32R)
            v_sb = const.tile([KT, NKT * DA], F32R)
            nc.sync.dma_start(out=qt_sb, in_=qt_d)
            nc.sync.dma_start(out=kt_sb, in_=kt_d)
            nc.sync.dma_start(out=qt2_sb, in_=qt2_d)
            nc.sync.dma_start(out=kt2_sb, in_=kt2_d)
            nc.sync.dma_start(
                out=v_sb.rearrange("p (t d) -> p t d", d=DA),
                in_=v_d.rearrange("(t p) d -> p t d", p=KT),
            )

            for qb in range(NQB):
                qsl = bass.ds(qb * QB, QB)
                out_ps = ops.tile([DA, QB], F32)
                pts = []
                for kp in range(NPAIR):
                    s_ps = sps.tile([KT, PAIR * QB], F32)
                    for j in range(PAIR):
                        kt = kp * PAIR + j
                        nc.tensor.matmul(
                            s_ps[:, bass.ts(j, QB)],
                            lhsT=kt_sb[:, bass.ts(kt, KT)],
                            rhs=qt_sb[:, qsl],
                            start=True,
                            stop=False,
                            skip_group_check=True,
                        )
                        nc.tensor.matmul(
                            s_ps[:, bass.ts(j, QB)],
                            lhsT=kt2_sb[:, bass.ts(kt, KT)],
                            rhs=qt2_sb[:, qsl],
                            start=False,
                            stop=True,
                            skip_group_check=True,
                        )
                    pt = ptp.tile([KT, PAIR * QB], F32R)
                    nc.scalar.activation(out=pt, in_=s_ps, func=EXP, scale=SCALE)
                    pts.append(pt)
                    for j in range(PAIR):
                        kt = kp * PAIR + j
                        nc.tensor.matmul(
                            out_ps,
                            lhsT=v_sb[:, bass.ts(kt, DA)],
                            rhs=pt[:, bass.ts(j, QB)],
                            start=(kt == 0),
                            stop=(kt == NKT - 1),
                            skip_group_check=True,
                        )

                # denominators -> reciprocal (exact fp32) -> replicate across
                # partitions via a DRAM-bounce broadcast
                out_sb = small.tile([DA, QB], F32)
                nc.scalar.copy(out=out_sb, in_=out_ps)
                recip_sb = small.tile([DA, QB], F32)
                nc.vector.reciprocal(
                    out=recip_sb[D : D + 1, :], in_=out_sb[D : D + 1, :]
                )
                recip_dr = drp.tile([1, QB], F32)
                nc.gpsimd.dma_start(out=recip_dr, in_=recip_sb[D : D + 1, :])
                rb_sb = small.tile([KT, QB], F32)
                nc.gpsimd.dma_start(
                    out=rb_sb, in_=recip_dr.partition_broadcast(KT)
                )

                # normalize attn tiles in place and store attn^T tiles
                for kp in range(NPAIR):
                    pt = pts[kp]
                    for j in range(PAIR):
                        nc.vector.tensor_mul(
                            pt[:, bass.ts(j, QB)], pt[:, bass.ts(j, QB)], rb_sb
                        )
                    # dram slice dims are [j, p, q]; reorder AP to [p, j, q]
                    # to match the SBUF source (partition dim must lead there)
                    dst = attn_d[kp * PAIR : (kp + 1) * PAIR, qb].transpose(
                        [1, 0, 2]
                    )
                    nc.sync.dma_start(
                        out=dst, in_=pt.rearrange("p (j q) -> p j q", q=QB)
                    )

                nc.gpsimd.dma_start(out=outd_d[:, qsl], in_=out_sb)

    nc.finalize()
    return nc


def kernel(queries, keys, values, valid_lens):
    from concourse.bass_utils import run_bass_kernel_spmd

    if "nc" not in _cache:
        _cache["nc"] = _build_bass()
    nc = _cache["nc"]

    queries = np.ascontiguousarray(queries, dtype=np.float32)
    keys = np.ascontiguousarray(keys, dtype=np.float32)
    values = np.ascontiguousarray(values, dtype=np.float32)
    valid_lens = np.asarray(valid_lens)

    pos = np.arange(S)
    in_maps = []
    for b in range(B):
        qt = queries[b].T.copy()  # [D, S]
        kt = keys[b].T.copy()
        qhi = _round_fp32r(qt)
        qlo = _round_fp32r(qt - qhi)
        khi = _round_fp32r(kt)
        klo = _round_fp32r(kt - khi)

        qt_aug = np.empty((DA, S), np.float32)
        qt_aug[:D] = qhi
        qt_aug[D] = 1.0
        kt_aug = np.empty((DA, S), np.float32)
        kt_aug[:D] = khi
        kt_aug[D] = np.where(pos < int(valid_lens[b]), 0.0, NEG_BIAS).astype(
            np.float32
        )
        qt2 = np.concatenate([qlo, qhi], axis=0)  # [2D, S]
        kt2 = np.concatenate([khi, klo], axis=0)
        v_aug = np.empty((S, DA), np.float32)
        v_aug[:, :D] = values[b]
        v_aug[:, D] = 1.0
        in_maps.append(
            {
                "qt_aug": qt_aug,
                "kt_aug": kt_aug,
                "qt2": qt2,
                "kt2": kt2,
                "v_aug": v_aug,
            }
        )

    trace = bool(_cache.get("trace"))
    res = run_bass_kernel_spmd(
        nc, in_maps, core_ids=list(range(B)), trace=trace
    )
    _cache["last_result"] = res

    out = np.empty((B, S, D), np.float32)
    attn = np.empty((B, S, S), np.float32)
    for b in range(B):
        r = res.results[b]
        outd = r["outd"]
        out[b] = (outd[:D] / outd[D : D + 1]).T
        # attn_t [kt, qb, k, q] -> attn[q, k]
        attn[b] = (
            r["attn_t"].transpose(1, 3, 0, 2).reshape(S, S)
        )
    return (out, attn)
